# revision 34
# baseline (speedup 1.0000x reference)
"""Trainium2 Bass kernel for nn_Cross_Attention (B=8, N=2048, D=768).

Math (per batch b):
    key   = softmax(t, axis=-1).T              (t in {x2, x3})
    query = softmax(t, axis=0)
    attn  = (x @ key^T) @ query = x @ KQ       with KQ [D, D]
    out   = f*(attn_1 @ W1^T + b1) + f*(attn_2 @ W2^T + b2) + x
          = x @ Msum + (x + f*(b1+b2))
    Msum  = f*(KQ_1 @ W1^T + KQ_2 @ W2^T)

KQ[d,d'] = KQ_raw[d,d']/S[d'] with KQ_raw = E^T diag(1/R) E SYMMETRIC,
so only the upper-triangle blocks of KQ_raw are computed (all six row
tiles accumulate in PSUM at once - no post-stream second gram pass; one
accumulation group per 2KB PSUM bank, 8 banks exactly) and the lower
blocks are PE-transposed mirrors of a bf16 staging copy.  Mirror rounds
rotate through the freed gram banks; for t=1 they are woven into the
Msum tile loop (Msum runs d=5..0, needing mirrors latest-first) so Msum
never waits.  Row tile 2 has no room for a ones column (its 513-wide
span would need a 2nd group in the same bank), so its colsum S2 is a
tiny post-gram pass: 8 DR matmuls with a ones lhsT -> S2 as a row
vector, transposed back to a column by one f32 matmul.  All heavy
matmuls run fp8 DoubleRow.

Distribution: pure data-parallel, batch b -> core b, no collectives.
x2/x3 stream 4 tokens per partition line (3 KB descriptors).
"""

import numpy as np
import ml_dtypes

import concourse.bass as bass
import concourse.tile as tile
from concourse import bacc
from concourse import mybir
from concourse.bass_utils import run_bass_kernel_spmd

F32 = mybir.dt.float32
BF16 = mybir.dt.bfloat16
FP8 = mybir.dt.float8e4

NP_FP8 = ml_dtypes.float8_e4m3
NP_BF16 = ml_dtypes.bfloat16

B = 8
P = 128
D = 768
DT = D // P    # 6 feature subtiles
NT = 16        # 128-token tiles
TG = 4         # tokens per partition line / token tiles per DMA group
NG = NT // TG  # 4 stream groups
GD = D + 1     # g1 width: 768 data cols + the ones column
# fp8 prescales (exact powers of two; cancelled in the output scale)
CR = 1024.0    # on E/R   (g1)
CS = 64.0      # on KQ    (kqt)
CST = 16.0     # on raw KQ staging for mirrors
CW = 16.0      # on f*W^T (w8)
SO = 1.0 / (CS * CW)
DR = mybir.MatmulPerfMode.DoubleRow
MUL = mybir.AluOpType.mult
ADD = mybir.AluOpType.add
COPY = mybir.ActivationFunctionType.Copy
EXP = mybir.ActivationFunctionType.Exp

# gram triangle geometry -----------------------------------------------------
# row tile a holds KQ_raw rows [a*128,(a+1)*128), cols [a*128,768) (+ones).
# G psum tiles, one accumulation group per bank:
#   G0 [P,769]: row0 cols 0:512 | cols 512:768 + S0 @768
#   G1 [P,641]: row1 cols 128:640 | cols 640:768 + S1 @640
#   G2 [P,897]: row2 cols 256:768 (no ones) | row3 cols 384:768 + S3 @896
#   G3 [P,641]: row4 cols 512:768 + S4 @256 | row5 cols 640:768 + S5 @640
# stage (bf16 sbuf) packs off-diagonal upper spans (row a, cols >=(a+1)*128):
STG_OFF = (0, 640, 1152, 1664, 1920)
STG_W = (640, 512, 512, 256, 128)   # b=2 spans cols 256:768 incl diagonal
# mirror transpose rounds (post t1-gram): (tag, ((dest row a, offset), ...))
# descending a: Msum runs d=5..0 and needs high-a mirrors first.
X_ROUNDS = (("G1", ((5, 0),)), ("G2", ((4, 0), (3, 512))),
            ("G3", ((2, 0), (1, 256))))


def build_nc():
    N = NT * P
    nc = bacc.Bacc()

    x2_d = nc.dram_tensor("x2", [N, D], FP8, kind="ExternalInput")
    x3_d = nc.dram_tensor("x3", [N, D], FP8, kind="ExternalInput")
    xt8_d = nc.dram_tensor("xt8", [D, N], FP8, kind="ExternalInput")  # x^T
    w8_d = nc.dram_tensor("w8", [2 * D, D], FP8, kind="ExternalInput")
    xfb_d = nc.dram_tensor("xfb", [N, D], BF16, kind="ExternalInput")
    id_d = nc.dram_tensor("ident", [P, P], BF16, kind="ExternalInput")
    out_d = nc.dram_tensor("out", [N, D], BF16, kind="ExternalOutput")

    # token n = g*512 + p*4 + j  ->  3 KB contiguous partition lines
    att_g = [
        x2_d.rearrange("(g p j) d -> g p j d", p=P, j=TG),
        x3_d.rearrange("(g p j) d -> g p j d", p=P, j=TG),
    ]
    xt8_r = xt8_d.rearrange("(c p) n -> p c n", p=P)
    w8_r = w8_d.rearrange("(t c p) j -> p t c j", p=P, c=DT)
    xfb_r = xfb_d.rearrange("(h t p) d -> h p t d", p=P, t=NT // 2)
    out_t = out_d.rearrange("(t p) d -> t p d", p=P)

    with tile.TileContext(nc) as tc:
        with (
            tc.tile_pool(name="consts", bufs=1) as consts,
            tc.tile_pool(name="gbuf", bufs=2) as gbuf,
            tc.tile_pool(name="stream", bufs=3) as stream,
            tc.tile_pool(name="stats", bufs=2) as stats,
            tc.tile_pool(name="obuf", bufs=4) as obufp,
            tc.tile_pool(name="acc", bufs=1, space="PSUM") as acc,
        ):
            ones = consts.tile([P, 2, P], FP8)
            nc.vector.memset(ones, 1.0)
            onef = consts.tile([1, 1], F32)
            nc.vector.memset(onef, 1.0)
            ident = consts.tile([P, P], BF16)
            kqt = [consts.tile([P, DT, D], FP8, name=f"kqt{t}") for t in range(2)]
            msum = consts.tile([P, DT, D], FP8)
            xt8 = consts.tile([P, DT, N], FP8)
            w8 = consts.tile([P, 2, DT, D], FP8)
            xfb = consts.tile([P, NT, D], BF16)
            mpart = consts.tile([P, DT, D], F32, name="mpart")

            # probe tiles for the scalar-engine dtype-rate experiment
            pf8 = consts.tile([P, 2], FP8)
            po8 = consts.tile([P, 2], FP8, name="po8")
            nc.vector.memset(pf8, 0.25)
            # tiny exp at t=0: pulls the ACT table load into the startup gap
            nc.scalar.activation(out=po8[:, 0:1], in_=pf8[:, 0:1], func=EXP)

            # PE warmup (p-state ramp); rides in the G0 psum slot
            warm = acc.tile([P, GD], F32, tag="G0", name="warm")
            for _ in range(5):
                nc.tensor.matmul(
                    warm[:, 0:P], ones, ones, start=True, stop=True, perf_mode=DR
                )

            # --- input DMAs: queue the whole schedule on the sync engine ---
            xi_t = {}
            for t in range(2):
                for g in range(NG):
                    xi = stream.tile([P, TG, D], FP8, tag="in", name=f"xi{t}_{g}")
                    xi_t[(t, g)] = xi
                    if t == 0 and g == 0:
                        nc.sync.dma_start(out=xi[:, 0:1, :], in_=att_g[t][g][:, 0:1, :])
                        nc.sync.dma_start(out=xi[:, 1:2, :], in_=att_g[t][g][:, 1:2, :])
                        nc.sync.dma_start(out=xi[:, 2:4, :], in_=att_g[t][g][:, 2:4, :])
                        nc.sync.dma_start(out=ident, in_=id_d[:, :])
                    else:
                        nc.sync.dma_start(out=xi, in_=att_g[t][g])
            nc.sync.dma_start(out=w8, in_=w8_r)
            nc.sync.dma_start(out=xt8, in_=xt8_r)
            nc.sync.dma_start(out=xfb[:, 0:8, :], in_=xfb_r[0])
            nc.sync.dma_start(out=xfb[:, 8:16, :], in_=xfb_r[1])

            # --- per-t state ---
            st = {}

            def emit_stream_group(t, g, skip_g3=False):
                """exp + g1 scale + triangle gram passes for stream group g."""
                if g == 0:
                    d = {}
                    d["g2"] = gbuf.tile([P, NT, D], FP8, tag="g2", name=f"g2_{t}")
                    d["g1"] = gbuf.tile([P, NT, GD], FP8, tag="g1", name=f"g1_{t}")
                    nc.gpsimd.memset(d["g1"][:, :, D:GD], 1.0)
                    d["rv"] = stats.tile([P, NT], F32, tag="rvec", name=f"rv{t}")
                    d["rvr"] = stats.tile([P, NT], F32, tag="rvr", name=f"rvr{t}")
                    d["srR"] = stats.tile([P, DT], F32, tag="srR", name=f"srR{t}")
                    d["srA"] = stats.tile([P, DT], F32, tag="srA", name=f"srA{t}")
                    d["srB"] = stats.tile([P, DT], F32, tag="srB", name=f"srB{t}")
                    d["stg"] = gbuf.tile([P, 2048], BF16, tag="stage",
                                         name=f"stg{t}")
                    d["s2r"] = stats.tile([1, P], F32, tag="s2r", name=f"s2r{t}")
                    d["G"] = [
                        acc.tile([P, GD], F32, tag="G0", name=f"G0_{t}"),
                        acc.tile([P, 641], F32, tag="G1", name=f"G1_{t}"),
                        acc.tile([P, 897], F32, tag="G2", name=f"G2_{t}"),
                    ]
                    if not skip_g3:
                        d["G3"] = acc.tile([P, 641], F32, tag="G3",
                                           name=f"G3_{t}")
                    st[t] = d
                d = st[t]
                g1, g2, rvec, rvr = d["g1"], d["g2"], d["rv"], d["rvr"]
                xi = xi_t[(t, g)]
                for j in range(TG):
                    i = g * TG + j
                    nc.scalar.activation(
                        out=g2[:, i, :], in_=xi[:, j, :], func=EXP,
                        accum_out=rvec[:, i : i + 1],
                    )
                for h in range(2):
                    i0 = g * TG + 2 * h
                    nc.vector.reciprocal(
                        rvr[:, i0 : i0 + 2], rvec[:, i0 : i0 + 2]
                    )
                    for i in (i0, i0 + 1):
                        nc.vector.tensor_scalar(
                            out=g1[:, i, 0:D], in0=g2[:, i, :],
                            scalar1=rvr[:, i : i + 1], scalar2=CR,
                            op0=MUL, op1=MUL,
                        )
                G0, G1t, G2t = d["G"]
                G3t = d.get("G3")
                for half in range(2):
                    ip = 2 * g + half
                    pr = slice(4 * g + 2 * half, 4 * g + 2 * half + 2)
                    s0, s1 = (ip == 0), (ip == 7)

                    def mm(out, c0, c1, r0, r1):
                        nc.tensor.matmul(
                            out, g2[:, pr, c0:c1], g1[:, pr, r0:r1],
                            start=s0, stop=s1, perf_mode=DR,
                        )

                    mm(G0[:, 0:512], 0, 128, 0, 512)
                    mm(G0[:, 512:769], 0, 128, 512, 769)
                    mm(G1t[:, 0:512], 128, 256, 128, 640)
                    mm(G1t[:, 512:641], 128, 256, 640, 769)
                    mm(G2t[:, 0:512], 256, 384, 256, 768)
                    mm(G2t[:, 512:897], 384, 512, 384, 769)
                    if not skip_g3:
                        mm(G3t[:, 0:257], 512, 640, 512, 769)
                        mm(G3t[:, 512:641], 640, 768, 640, 769)

            # (S col, scaled-drain src, kqt row a, kqt col, stage idx or None)
            def plans(t):
                G0, G1t, G2t = st[t]["G"]
                G3t = st[t]["G3"]
                return [
                    (G0[:, 768:769], G0[:, 0:D], 0, 0, 0),
                    (G1t[:, 640:641], G1t[:, 0:640], 1, 128, 1),
                    (None, G2t[:, 0:512], 2, 256, 2),
                    (G2t[:, 896:897], G2t[:, 512:896], 3, 384, 3),
                    (G3t[:, 256:257], G3t[:, 0:256], 4, 512, 4),
                    (G3t[:, 640:641], G3t[:, 512:640], 5, 640, None),
                ]

            def stage_src(t, b):
                G0, G1t, G2t = st[t]["G"]
                G3t = st[t]["G3"]
                return [G0[:, 128:768], G1t[:, 128:640], G2t[:, 0:512],
                        G2t[:, 640:896], G3t[:, 128:256]][b]

            def emit_stage(t, b, eng="dve"):
                dst = st[t]["stg"][:, STG_OFF[b] : STG_OFF[b] + STG_W[b]]
                if eng == "dve":
                    nc.vector.tensor_scalar(
                        out=dst, in0=stage_src(t, b),
                        scalar1=CST / CR, scalar2=1.0, op0=MUL, op1=MUL,
                    )
                elif eng == "gpsimd":
                    nc.gpsimd.tensor_scalar(
                        out=dst, in0=stage_src(t, b),
                        scalar1=CST / CR, scalar2=1.0, op0=MUL, op1=MUL,
                    )
                else:
                    nc.scalar.activation(
                        out=dst, in_=stage_src(t, b), func=COPY, scale=CST / CR
                    )

            def emit_row(t, k, eng, do_stage=True):
                """recip + scaled drain (+ stage drain) for row k."""
                d = st[t]
                scol, src, a, c0, b = plans(t)[k]
                if scol is not None:
                    nc.vector.reciprocal(d["srR"][:, k : k + 1], scol)
                w = src.shape[-1]
                dst = kqt[t][:, a, c0 : c0 + w]
                if eng == "dve":
                    nc.vector.tensor_scalar(
                        out=dst, in0=src, scalar1=d["srR"][:, k : k + 1],
                        scalar2=CS / CR, op0=MUL, op1=MUL,
                    )
                elif eng == "gpsimd":
                    nc.gpsimd.tensor_scalar(
                        out=dst, in0=src, scalar1=d["srR"][:, k : k + 1],
                        scalar2=CS / CR, op0=MUL, op1=MUL,
                    )
                else:
                    nc.vector.tensor_scalar(
                        out=d["srA"][:, k : k + 1], in0=d["srR"][:, k : k + 1],
                        scalar1=CS / CR, scalar2=1.0, op0=MUL, op1=MUL,
                    )
                    nc.scalar.activation(
                        out=dst, in_=src, func=COPY,
                        scale=d["srA"][:, k : k + 1],
                    )
                if b is not None and do_stage:
                    emit_stage(t, b)

            def emit_g3_gram(t):
                d = st[t]
                d["G3"] = acc.tile([P, 641], F32, tag="G3", name=f"G3_{t}")
                G3t = d["G3"]
                g1, g2 = d["g1"], d["g2"]
                for ip in range(8):
                    pr = slice(2 * ip, 2 * ip + 2)
                    s0, s1 = (ip == 0), (ip == 7)
                    nc.tensor.matmul(
                        G3t[:, 0:257], g2[:, pr, 512:640], g1[:, pr, 512:769],
                        start=s0, stop=s1, perf_mode=DR,
                    )
                    nc.tensor.matmul(
                        G3t[:, 512:641], g2[:, pr, 640:768], g1[:, pr, 640:769],
                        start=s0, stop=s1, perf_mode=DR,
                    )

            def emit_mt0(dd, eng):
                # Msum t0-half for output tile dd -> f32 partial in SBUF
                mt_ps = acc.tile([P, D], F32, tag="G3", name=f"mt{dd}")
                for dpp in range(3):
                    lhsT = kqt[0][:, 2 * dpp : 2 * dpp + 2,
                                  dd * P : (dd + 1) * P]
                    for off, sz in ((0, 512), (512, 256)):
                        nc.tensor.matmul(
                            mt_ps[:, off : off + sz], lhsT,
                            w8[:, 0, 2 * dpp : 2 * dpp + 2, off : off + sz],
                            start=(dpp == 0), stop=(dpp == 2), perf_mode=DR,
                        )
                e = nc.vector if eng == "dve" else nc.gpsimd
                e.tensor_copy(mpart[:, dd, :], mt_ps)

            def emit_row2_final(t, eng="dve"):
                # kqt row2 upper from the bf16 stage (needs 1/S2 from s2 pass)
                d = st[t]
                e = nc.vector if eng == "dve" else nc.gpsimd
                e.tensor_scalar(
                    out=kqt[t][:, 2, 256:D],
                    in0=d["stg"][:, STG_OFF[2] : STG_OFF[2] + 512],
                    scalar1=d["srR"][:, 2:3], scalar2=CS / CST,
                    op0=MUL, op1=MUL,
                )

            def emit_s2_pass(t):
                """row2 colsum: ones^T @ g2 cols -> s2 row; transpose; recip."""
                d = st[t]
                s2_ps = acc.tile([1, P], F32, tag="G0", name=f"s2ps{t}")
                g2 = d["g2"]
                for ip in range(8):
                    pr = slice(2 * ip, 2 * ip + 2)
                    nc.tensor.matmul(
                        s2_ps, ones[:, :, 0:1], g2[:, pr, 256:384],
                        start=(ip == 0), stop=(ip == 7), perf_mode=DR,
                    )
                nc.vector.tensor_copy(d["s2r"], s2_ps)
                s2c = acc.tile([P, 1], F32, tag="G1", name=f"s2c{t}")
                nc.tensor.matmul(s2c, d["s2r"], onef, start=True, stop=True)
                nc.vector.reciprocal(d["srR"][:, 2:3], s2c)

            def emit_round(t, r, eng):
                """transpose round r + its mirror drains."""
                d = st[t]
                tag, blocks = X_ROUNDS[r]
                X = acc.tile([P, 1024], BF16, tag=tag, name=f"X{t}_{r}")
                stg = d["stg"]
                for a, xoff in blocks:
                    for b in range(a):
                        # span b starts at col (b+1)*128, except b=2 at col 256
                        s0 = STG_OFF[b] + (a - b - 1) * P + (P if b == 2 else 0)
                        nc.tensor.transpose(
                            X[:, xoff + b * P : xoff + (b + 1) * P],
                            stg[:, s0 : s0 + P], ident,
                        )
                for a, xoff in blocks:
                    dst = kqt[t][:, a, 0 : a * P]
                    src = X[:, xoff : xoff + a * P]
                    if eng == "gpsimd":
                        nc.gpsimd.tensor_scalar(
                            out=dst, in0=src, scalar1=d["srR"][:, a : a + 1],
                            scalar2=CS / CST, op0=MUL, op1=MUL,
                        )
                    elif eng == "dve":
                        nc.vector.tensor_scalar(
                            out=dst, in0=src, scalar1=d["srR"][:, a : a + 1],
                            scalar2=CS / CST, op0=MUL, op1=MUL,
                        )
                    else:
                        nc.vector.tensor_scalar(
                            out=d["srB"][:, a : a + 1],
                            in0=d["srR"][:, a : a + 1],
                            scalar1=CS / CST, scalar2=1.0, op0=MUL, op1=MUL,
                        )
                        nc.scalar.activation(
                            out=dst, in_=src, func=COPY,
                            scale=d["srB"][:, a : a + 1],
                        )

            # ------------------- schedule -------------------
            for g in range(NG):
                emit_stream_group(0, g)
            # t0 drains (DVE) woven with t1's stream groups: free banks in
            # tag order G0..G3 so t1's gram mms unblock in emission order
            emit_row(0, 0, "dve")
            emit_stream_group(1, 0)
            emit_row(0, 1, "dve")
            emit_stream_group(1, 1)
            emit_stage(0, 2)
            emit_row(0, 3, "dve")
            emit_stream_group(1, 2)
            emit_row(0, 4, "dve")
            emit_row(0, 5, "dve")
            emit_stream_group(1, 3)

            # t1 drains: rows on scalar (free post-exp), stages on DVE
            emit_row(1, 0, "scalar")
            emit_row(1, 1, "scalar")
            emit_stage(1, 2)
            emit_row(1, 3, "scalar")
            emit_row(1, 4, "scalar")
            emit_row(1, 5, "scalar")

            # s2 colsum passes (bank G0 freed first), then row2 finals
            emit_s2_pass(0)
            emit_s2_pass(1)
            emit_row2_final(0)
            emit_row2_final(1)

            # --- Msum, output tiles d = 5..0 (mirror-light first); mirror
            # rounds woven in so M_d never waits on a mirror ---
            CHUNKS = ((0, 512), (512, 256))
            m_tags = {5: "G0", 4: "G1", 3: "G2", 2: "G0", 1: "G1", 0: "G2"}
            for d in (5, 4, 3, 2, 1, 0):
                m_ps = acc.tile([P, D], F32, tag=m_tags[d], name=f"m{d}")
                dpps = (0, 2, 1) if d == 5 else (0, 1, 2)
                for dpp in dpps:
                    for t in range(2):
                        lhsT = kqt[t][:, 2 * dpp : 2 * dpp + 2, d * P : (d + 1) * P]
                        for off, sz in CHUNKS:
                            nc.tensor.matmul(
                                m_ps[:, off : off + sz], lhsT,
                                w8[:, t, 2 * dpp : 2 * dpp + 2, off : off + sz],
                                start=(t == 0 and dpp == dpps[0]),
                                stop=(t == 1 and dpp == dpps[-1]),
                                perf_mode=DR,
                            )
                nc.scalar.activation(out=msum[:, d, :], in_=m_ps, func=COPY)
                if d == 5:
                    emit_round(0, 0, "scalar")   # a5 mirrors
                    emit_round(1, 0, "scalar")
                elif d == 4:
                    emit_round(0, 1, "scalar")   # a4, a3
                    emit_round(1, 1, "scalar")
                elif d == 3:
                    emit_round(0, 2, "dve")      # a2, a1
                    emit_round(1, 2, "dve")

            # --- y = x @ Msum; out = y*SO + (x + fb) ---
            y_tags = ("G0", "G1", "G2")
            for i in range(NT):
                y_ps = acc.tile([P, D], F32, tag=y_tags[i % 3], name=f"y{i}")
                ob = obufp.tile([P, D], BF16, tag="out", name=f"ob{i}")
                if i < NT - 1:
                    for cp in (2, 1, 0):
                        lhsT = xt8[:, 2 * cp : 2 * cp + 2, i * P : (i + 1) * P]
                        for off, sz in CHUNKS:
                            nc.tensor.matmul(
                                y_ps[:, off : off + sz], lhsT,
                                msum[:, 2 * cp : 2 * cp + 2, off : off + sz],
                                start=(cp == 2), stop=(cp == 0), perf_mode=DR,
                            )
                    nc.vector.scalar_tensor_tensor(
                        out=ob, in0=y_ps, scalar=SO,
                        in1=xfb[:, i, :], op0=MUL, op1=ADD,
                    )
                    eng = nc.scalar if (i % 2 == 0) else nc.sync
                    eng.dma_start(out=out_t[i], in_=ob)
                else:
                    # last tile: drain per 1-bank chunk to shrink the tail
                    for off, sz in CHUNKS:
                        for cp in (2, 1, 0):
                            lhsT = xt8[:, 2 * cp : 2 * cp + 2, i * P : (i + 1) * P]
                            nc.tensor.matmul(
                                y_ps[:, off : off + sz], lhsT,
                                msum[:, 2 * cp : 2 * cp + 2, off : off + sz],
                                start=(cp == 2), stop=(cp == 0), perf_mode=DR,
                            )
                        nc.vector.scalar_tensor_tensor(
                            out=ob[:, off : off + sz], in0=y_ps[:, off : off + sz],
                            scalar=SO, in1=xfb[:, i, off : off + sz],
                            op0=MUL, op1=ADD,
                        )
                        eng = nc.scalar if off == 0 else nc.sync
                        eng.dma_start(
                            out=out_t[i][:, off : off + sz],
                            in_=ob[:, off : off + sz],
                        )

    nc.compile()
    return nc


def prep_inputs(inputs):
    x = np.asarray(inputs["x"], dtype=np.float32)
    x2 = np.asarray(inputs["x2"], dtype=np.float32)
    x3 = np.asarray(inputs["x3"], dtype=np.float32)
    W1 = np.asarray(inputs["W1"], dtype=np.float32)
    b1 = np.asarray(inputs["b1"], dtype=np.float32)
    W2 = np.asarray(inputs["W2"], dtype=np.float32)
    b2 = np.asarray(inputs["b2"], dtype=np.float32)
    w = np.asarray(inputs["w"], dtype=np.float32)

    f = 1.0 / (1.0 + np.exp(-float(w.reshape(-1)[0])))
    w8 = np.concatenate(
        [(f * CW * W1).T, (f * CW * W2).T], axis=0
    ).astype(NP_FP8)
    fb = (f * (b1 + b2)).astype(np.float32)

    x2_8 = x2.astype(NP_FP8)
    x3_8 = x3.astype(NP_FP8)
    xfb = (x + fb[None, None, :]).astype(NP_BF16)
    ident = np.eye(P, dtype=NP_BF16)
    return [
        {
            "x2": np.ascontiguousarray(x2_8[b]),
            "x3": np.ascontiguousarray(x3_8[b]),
            "xt8": np.ascontiguousarray(x[b].T).astype(NP_FP8),
            "w8": w8,
            "xfb": np.ascontiguousarray(xfb[b]),
            "ident": ident,
        }
        for b in range(B)
    ]


_NC = None


def kernel(**inputs) -> np.ndarray:
    global _NC
    if _NC is None:
        _NC = build_nc()
    in_maps = prep_inputs(inputs)
    res = run_bass_kernel_spmd(_NC, in_maps, list(range(B)))
    return np.stack(
        [res.results[b]["out"] for b in range(B)], axis=0
    ).astype(np.float32)


# revision 36
# speedup vs baseline: 1.0346x; 1.0346x over previous
"""Trainium2 Bass kernel for nn_Cross_Attention (B=8, N=2048, D=768).

Math (per batch b):
    key   = softmax(t, axis=-1).T              (t in {x2, x3})
    query = softmax(t, axis=0)
    attn  = (x @ key^T) @ query = x @ KQ       with KQ [D, D]
    out   = f*(attn_1 @ W1^T + b1) + f*(attn_2 @ W2^T + b2) + x
          = x @ Msum + (x + f*(b1+b2))
    Msum  = f*(KQ_1 @ W1^T + KQ_2 @ W2^T)

KQ[d,d'] = KQ_raw[d,d']/S[d'] with KQ_raw = E^T diag(1/R) E SYMMETRIC,
so only the upper-triangle blocks of KQ_raw are computed (all six row
tiles accumulate in PSUM at once - no post-stream second gram pass; one
accumulation group per 2KB PSUM bank, 8 banks exactly) and the lower
blocks are PE-transposed mirrors of a bf16 staging copy.  Mirror rounds
rotate through the freed gram banks; for t=1 they are woven into the
Msum tile loop (Msum runs d=5..0, needing mirrors latest-first) so Msum
never waits.  Row tile 2 has no room for a ones column (its 513-wide
span would need a 2nd group in the same bank), so its colsum S2 is a
tiny post-gram pass: 8 DR matmuls with a ones lhsT -> S2 as a row
vector, transposed back to a column by one f32 matmul.  All heavy
matmuls run fp8 DoubleRow.

Distribution: pure data-parallel, batch b -> core b, no collectives.
x2/x3 stream 4 tokens per partition line (3 KB descriptors).
"""

import numpy as np
import ml_dtypes

import concourse.bass as bass
import concourse.tile as tile
from concourse import bacc
from concourse import mybir
from concourse.bass_utils import run_bass_kernel_spmd

F32 = mybir.dt.float32
BF16 = mybir.dt.bfloat16
FP8 = mybir.dt.float8e4

NP_FP8 = ml_dtypes.float8_e4m3
NP_BF16 = ml_dtypes.bfloat16

B = 8
P = 128
D = 768
DT = D // P    # 6 feature subtiles
NT = 16        # 128-token tiles
TG = 4         # tokens per partition line / token tiles per DMA group
NG = NT // TG  # 4 stream groups
GD = D + 1     # g1 width: 768 data cols + the ones column
# fp8 prescales (exact powers of two; cancelled in the output scale)
CR = 1024.0    # on E/R   (g1)
CS = 64.0      # on KQ    (kqt)
CST = 16.0     # on raw KQ staging for mirrors
CW = 16.0      # on f*W^T (w8)
SO = 1.0 / (CS * CW)
DR = mybir.MatmulPerfMode.DoubleRow
MUL = mybir.AluOpType.mult
ADD = mybir.AluOpType.add
COPY = mybir.ActivationFunctionType.Copy
EXP = mybir.ActivationFunctionType.Exp

# gram triangle geometry -----------------------------------------------------
# row tile a holds KQ_raw rows [a*128,(a+1)*128), cols [a*128,768) (+ones).
# G psum tiles, one accumulation group per bank:
#   G0 [P,769]: row0 cols 0:512 | cols 512:768 + S0 @768
#   G1 [P,641]: row1 cols 128:640 | cols 640:768 + S1 @640
#   G2 [P,897]: row2 cols 256:768 (no ones) | row3 cols 384:768 + S3 @896
#   G3 [P,641]: row4 cols 512:768 + S4 @256 | row5 cols 640:768 + S5 @640
# stage (bf16 sbuf) packs off-diagonal upper spans (row a, cols >=(a+1)*128):
STG_OFF = (0, 640, 1152, 1664, 1920)
STG_W = (640, 512, 512, 256, 128)   # b=2 spans cols 256:768 incl diagonal
# mirror transpose rounds (post t1-gram): (tag, ((dest row a, offset), ...))
# descending a: Msum runs d=5..0 and needs high-a mirrors first.
X_ROUNDS = (("G1", ((5, 0),)), ("G2", ((4, 0), (3, 512))),
            ("G3", ((2, 0), (1, 256))))


def build_nc():
    N = NT * P
    nc = bacc.Bacc()

    x2_d = nc.dram_tensor("x2", [N, D], FP8, kind="ExternalInput")
    x3_d = nc.dram_tensor("x3", [N, D], FP8, kind="ExternalInput")
    xt8_d = nc.dram_tensor("xt8", [D, N], FP8, kind="ExternalInput")  # x^T
    w8_d = nc.dram_tensor("w8", [2 * D, D], FP8, kind="ExternalInput")
    xfb_d = nc.dram_tensor("xfb", [N, D], BF16, kind="ExternalInput")
    id_d = nc.dram_tensor("ident", [P, P], BF16, kind="ExternalInput")
    out_d = nc.dram_tensor("out", [N, D], BF16, kind="ExternalOutput")

    # token n = g*512 + p*4 + j  ->  3 KB contiguous partition lines
    att_g = [
        x2_d.rearrange("(g p j) d -> g p j d", p=P, j=TG),
        x3_d.rearrange("(g p j) d -> g p j d", p=P, j=TG),
    ]
    xt8_r = xt8_d.rearrange("(c p) n -> p c n", p=P)
    w8_r = w8_d.rearrange("(t c p) j -> p t c j", p=P, c=DT)
    xfb_r = xfb_d.rearrange("(h t p) d -> h p t d", p=P, t=NT // 2)
    out_t = out_d.rearrange("(t p) d -> t p d", p=P)

    with tile.TileContext(nc) as tc:
        with (
            tc.tile_pool(name="consts", bufs=1) as consts,
            tc.tile_pool(name="gbuf", bufs=2) as gbuf,
            tc.tile_pool(name="stream", bufs=3) as stream,
            tc.tile_pool(name="stats", bufs=2) as stats,
            tc.tile_pool(name="obuf", bufs=4) as obufp,
            tc.tile_pool(name="acc", bufs=1, space="PSUM") as acc,
        ):
            ones = consts.tile([P, 2, P], FP8)
            nc.vector.memset(ones, 1.0)
            onef = consts.tile([1, 1], F32)
            nc.vector.memset(onef, 1.0)
            ident = consts.tile([P, P], BF16)
            kqt = [consts.tile([P, DT, D], FP8, name=f"kqt{t}") for t in range(2)]
            msum = consts.tile([P, DT, D], FP8)
            xt8 = consts.tile([P, DT, N], FP8)
            w8 = consts.tile([P, 2, DT, D], FP8)
            xfb = consts.tile([P, NT, D], BF16)
            mpart = consts.tile([P, DT, D], F32, name="mpart")

            # probe tiles for the scalar-engine dtype-rate experiment
            pf8 = consts.tile([P, 2], FP8)
            po8 = consts.tile([P, 2], FP8, name="po8")
            nc.vector.memset(pf8, 0.25)
            # tiny exp at t=0: pulls the ACT table load into the startup gap
            nc.scalar.activation(out=po8[:, 0:1], in_=pf8[:, 0:1], func=EXP)

            # PE warmup (p-state ramp); rides in the G0 psum slot
            warm = acc.tile([P, GD], F32, tag="G0", name="warm")
            for _ in range(5):
                nc.tensor.matmul(
                    warm[:, 0:P], ones, ones, start=True, stop=True, perf_mode=DR
                )

            # --- input DMAs: queue the whole schedule on the sync engine ---
            xi_t = {}
            for t in range(2):
                for g in range(NG):
                    xi = stream.tile([P, TG, D], FP8, tag="in", name=f"xi{t}_{g}")
                    xi_t[(t, g)] = xi
                    if t == 0 and g == 0:
                        nc.sync.dma_start(out=xi[:, 0:1, :], in_=att_g[t][g][:, 0:1, :])
                        nc.sync.dma_start(out=xi[:, 1:2, :], in_=att_g[t][g][:, 1:2, :])
                        nc.sync.dma_start(out=xi[:, 2:4, :], in_=att_g[t][g][:, 2:4, :])
                        nc.sync.dma_start(out=ident, in_=id_d[:, :])
                    else:
                        nc.sync.dma_start(out=xi, in_=att_g[t][g])
            nc.sync.dma_start(out=w8, in_=w8_r)
            nc.sync.dma_start(out=xt8, in_=xt8_r)
            nc.sync.dma_start(out=xfb[:, 0:8, :], in_=xfb_r[0])
            nc.sync.dma_start(out=xfb[:, 8:16, :], in_=xfb_r[1])

            # --- per-t state ---
            st = {}

            def emit_stream_group(t, g, skip_g3=False):
                """exp + g1 scale + triangle gram passes for stream group g."""
                if g == 0:
                    d = {}
                    d["g2"] = gbuf.tile([P, NT, D], FP8, tag="g2", name=f"g2_{t}")
                    d["g1"] = gbuf.tile([P, NT, GD], FP8, tag="g1", name=f"g1_{t}")
                    nc.gpsimd.memset(d["g1"][:, :, D:GD], 1.0)
                    d["rv"] = stats.tile([P, NT], F32, tag="rvec", name=f"rv{t}")
                    d["rvr"] = stats.tile([P, NT], F32, tag="rvr", name=f"rvr{t}")
                    d["srR"] = stats.tile([P, DT], F32, tag="srR", name=f"srR{t}")
                    d["srA"] = stats.tile([P, DT], F32, tag="srA", name=f"srA{t}")
                    d["srB"] = stats.tile([P, DT], F32, tag="srB", name=f"srB{t}")
                    d["stg"] = gbuf.tile([P, 2048], BF16, tag="stage",
                                         name=f"stg{t}")
                    d["s2r"] = stats.tile([1, P], F32, tag="s2r", name=f"s2r{t}")
                    d["G"] = [
                        acc.tile([P, GD], F32, tag="G0", name=f"G0_{t}"),
                        acc.tile([P, 641], F32, tag="G1", name=f"G1_{t}"),
                        acc.tile([P, 897], F32, tag="G2", name=f"G2_{t}"),
                    ]
                    if not skip_g3:
                        d["G3"] = acc.tile([P, 641], F32, tag="G3",
                                           name=f"G3_{t}")
                    st[t] = d
                d = st[t]
                g1, g2, rvec, rvr = d["g1"], d["g2"], d["rv"], d["rvr"]
                xi = xi_t[(t, g)]
                for j in range(TG):
                    i = g * TG + j
                    nc.scalar.activation(
                        out=g2[:, i, :], in_=xi[:, j, :], func=EXP,
                        accum_out=rvec[:, i : i + 1],
                    )
                for h in range(2):
                    i0 = g * TG + 2 * h
                    nc.vector.reciprocal(
                        rvr[:, i0 : i0 + 2], rvec[:, i0 : i0 + 2]
                    )
                    for i in (i0, i0 + 1):
                        nc.vector.tensor_scalar(
                            out=g1[:, i, 0:D], in0=g2[:, i, :],
                            scalar1=rvr[:, i : i + 1], scalar2=CR,
                            op0=MUL, op1=MUL,
                        )
                G0, G1t, G2t = d["G"]
                G3t = d.get("G3")
                for half in range(2):
                    ip = 2 * g + half
                    pr = slice(4 * g + 2 * half, 4 * g + 2 * half + 2)
                    s0, s1 = (ip == 0), (ip == 7)

                    def mm(out, c0, c1, r0, r1):
                        nc.tensor.matmul(
                            out, g2[:, pr, c0:c1], g1[:, pr, r0:r1],
                            start=s0, stop=s1, perf_mode=DR,
                        )

                    mm(G0[:, 0:512], 0, 128, 0, 512)
                    mm(G0[:, 512:769], 0, 128, 512, 769)
                    mm(G1t[:, 0:512], 128, 256, 128, 640)
                    mm(G1t[:, 512:641], 128, 256, 640, 769)
                    mm(G2t[:, 0:512], 256, 384, 256, 768)
                    mm(G2t[:, 512:897], 384, 512, 384, 769)
                    if not skip_g3:
                        mm(G3t[:, 0:257], 512, 640, 512, 769)
                        mm(G3t[:, 512:641], 640, 768, 640, 769)

            # (S col, scaled-drain src, kqt row a, kqt col, stage idx or None)
            def plans(t):
                G0, G1t, G2t = st[t]["G"]
                G3t = st[t]["G3"]
                return [
                    (G0[:, 768:769], G0[:, 0:D], 0, 0, 0),
                    (G1t[:, 640:641], G1t[:, 0:640], 1, 128, 1),
                    (None, G2t[:, 0:512], 2, 256, 2),
                    (G2t[:, 896:897], G2t[:, 512:896], 3, 384, 3),
                    (G3t[:, 256:257], G3t[:, 0:256], 4, 512, 4),
                    (G3t[:, 640:641], G3t[:, 512:640], 5, 640, None),
                ]

            def stage_src(t, b):
                G0, G1t, G2t = st[t]["G"]
                G3t = st[t]["G3"]
                return [G0[:, 128:768], G1t[:, 128:640], G2t[:, 0:512],
                        G2t[:, 640:896], G3t[:, 128:256]][b]

            def emit_stage(t, b, eng="dve"):
                dst = st[t]["stg"][:, STG_OFF[b] : STG_OFF[b] + STG_W[b]]
                if eng == "dve":
                    nc.vector.tensor_scalar(
                        out=dst, in0=stage_src(t, b),
                        scalar1=CST / CR, scalar2=1.0, op0=MUL, op1=MUL,
                    )
                elif eng == "gpsimd":
                    nc.gpsimd.tensor_scalar(
                        out=dst, in0=stage_src(t, b),
                        scalar1=CST / CR, scalar2=1.0, op0=MUL, op1=MUL,
                    )
                else:
                    nc.scalar.activation(
                        out=dst, in_=stage_src(t, b), func=COPY, scale=CST / CR
                    )

            def emit_row(t, k, eng, do_stage=True):
                """recip + scaled drain (+ stage drain) for row k."""
                d = st[t]
                scol, src, a, c0, b = plans(t)[k]
                if scol is not None:
                    nc.vector.reciprocal(d["srR"][:, k : k + 1], scol)
                w = src.shape[-1]
                dst = kqt[t][:, a, c0 : c0 + w]
                if eng == "dve":
                    nc.vector.tensor_scalar(
                        out=dst, in0=src, scalar1=d["srR"][:, k : k + 1],
                        scalar2=CS / CR, op0=MUL, op1=MUL,
                    )
                elif eng == "gpsimd":
                    nc.gpsimd.tensor_scalar(
                        out=dst, in0=src, scalar1=d["srR"][:, k : k + 1],
                        scalar2=CS / CR, op0=MUL, op1=MUL,
                    )
                else:
                    nc.vector.tensor_scalar(
                        out=d["srA"][:, k : k + 1], in0=d["srR"][:, k : k + 1],
                        scalar1=CS / CR, scalar2=1.0, op0=MUL, op1=MUL,
                    )
                    nc.scalar.activation(
                        out=dst, in_=src, func=COPY,
                        scale=d["srA"][:, k : k + 1],
                    )
                if b is not None and do_stage:
                    emit_stage(t, b)

            def emit_g3_gram(t):
                d = st[t]
                d["G3"] = acc.tile([P, 641], F32, tag="G3", name=f"G3_{t}")
                G3t = d["G3"]
                g1, g2 = d["g1"], d["g2"]
                for ip in range(8):
                    pr = slice(2 * ip, 2 * ip + 2)
                    s0, s1 = (ip == 0), (ip == 7)
                    nc.tensor.matmul(
                        G3t[:, 0:257], g2[:, pr, 512:640], g1[:, pr, 512:769],
                        start=s0, stop=s1, perf_mode=DR,
                    )
                    nc.tensor.matmul(
                        G3t[:, 512:641], g2[:, pr, 640:768], g1[:, pr, 640:769],
                        start=s0, stop=s1, perf_mode=DR,
                    )

            def emit_mt0(dd, eng):
                # Msum t0-half for output tile dd -> f32 partial in SBUF
                mt_ps = acc.tile([P, D], F32, tag="G3", name=f"mt{dd}")
                for dpp in range(3):
                    lhsT = kqt[0][:, 2 * dpp : 2 * dpp + 2,
                                  dd * P : (dd + 1) * P]
                    for off, sz in ((0, 512), (512, 256)):
                        nc.tensor.matmul(
                            mt_ps[:, off : off + sz], lhsT,
                            w8[:, 0, 2 * dpp : 2 * dpp + 2, off : off + sz],
                            start=(dpp == 0), stop=(dpp == 2), perf_mode=DR,
                        )
                e = nc.vector if eng == "dve" else nc.gpsimd
                e.tensor_copy(mpart[:, dd, :], mt_ps)

            def emit_row2_final(t, eng="dve"):
                # kqt row2 upper from the bf16 stage (needs 1/S2 from s2 pass)
                d = st[t]
                e = nc.vector if eng == "dve" else nc.gpsimd
                e.tensor_scalar(
                    out=kqt[t][:, 2, 256:D],
                    in0=d["stg"][:, STG_OFF[2] : STG_OFF[2] + 512],
                    scalar1=d["srR"][:, 2:3], scalar2=CS / CST,
                    op0=MUL, op1=MUL,
                )

            def emit_s2_pass(t):
                """row2 colsum: ones^T @ g2 cols -> s2 row; transpose; recip."""
                d = st[t]
                s2_ps = acc.tile([1, P], F32, tag="G0", name=f"s2ps{t}")
                g2 = d["g2"]
                for ip in range(8):
                    pr = slice(2 * ip, 2 * ip + 2)
                    nc.tensor.matmul(
                        s2_ps, ones[:, :, 0:1], g2[:, pr, 256:384],
                        start=(ip == 0), stop=(ip == 7), perf_mode=DR,
                    )
                nc.vector.tensor_copy(d["s2r"], s2_ps)
                s2c = acc.tile([P, 1], F32, tag="G0", name=f"s2c{t}")
                nc.tensor.matmul(s2c, d["s2r"], onef, start=True, stop=True)
                nc.vector.reciprocal(d["srR"][:, 2:3], s2c)

            def emit_round(t, r, eng):
                """transpose round r + its mirror drains."""
                d = st[t]
                tag, blocks = X_ROUNDS[r]
                X = acc.tile([P, 1024], BF16, tag=tag, name=f"X{t}_{r}")
                stg = d["stg"]
                for a, xoff in blocks:
                    for b in range(a):
                        # span b starts at col (b+1)*128, except b=2 at col 256
                        s0 = STG_OFF[b] + (a - b - 1) * P + (P if b == 2 else 0)
                        nc.tensor.transpose(
                            X[:, xoff + b * P : xoff + (b + 1) * P],
                            stg[:, s0 : s0 + P], ident,
                        )
                for a, xoff in blocks:
                    dst = kqt[t][:, a, 0 : a * P]
                    src = X[:, xoff : xoff + a * P]
                    if eng == "gpsimd":
                        nc.gpsimd.tensor_scalar(
                            out=dst, in0=src, scalar1=d["srR"][:, a : a + 1],
                            scalar2=CS / CST, op0=MUL, op1=MUL,
                        )
                    elif eng == "dve":
                        nc.vector.tensor_scalar(
                            out=dst, in0=src, scalar1=d["srR"][:, a : a + 1],
                            scalar2=CS / CST, op0=MUL, op1=MUL,
                        )
                    else:
                        nc.vector.tensor_scalar(
                            out=d["srB"][:, a : a + 1],
                            in0=d["srR"][:, a : a + 1],
                            scalar1=CS / CST, scalar2=1.0, op0=MUL, op1=MUL,
                        )
                        nc.scalar.activation(
                            out=dst, in_=src, func=COPY,
                            scale=d["srB"][:, a : a + 1],
                        )

            # ------------------- schedule -------------------
            for g in range(NG):
                emit_stream_group(0, g)
            # t0 drains (DVE) woven with t1's stream groups: free banks in
            # tag order G0..G3 so t1's gram mms unblock in emission order
            emit_row(0, 0, "dve")
            emit_stream_group(1, 0)
            emit_row(0, 1, "dve")
            emit_stream_group(1, 1)
            emit_stage(0, 2)
            emit_row(0, 3, "dve")
            emit_stream_group(1, 2)
            emit_row(0, 4, "dve")
            emit_row(0, 5, "dve")
            emit_stream_group(1, 3)

            # t1 drains: rows on scalar (free post-exp), stages on DVE
            emit_row(1, 0, "scalar")
            emit_row(1, 1, "scalar")
            emit_stage(1, 2)
            emit_row(1, 3, "scalar")
            emit_row(1, 4, "scalar")
            emit_row(1, 5, "scalar")

            # s2 colsum passes (bank G0 freed first), then row2 finals
            emit_s2_pass(0)
            emit_s2_pass(1)
            emit_row2_final(0)
            emit_row2_final(1)

            # --- Msum, output tiles d = 5..0 (mirror-light first); mirror
            # rounds woven in so M_d never waits on a mirror ---
            CHUNKS = ((0, 512), (512, 256))
            m_tags = {5: "G0", 4: "G1", 3: "G2", 2: "G0", 1: "G1", 0: "G2"}
            for d in (5, 4, 3, 2, 1, 0):
                if d == 5:
                    emit_round(0, 0, "scalar")   # a5 mirrors
                    emit_round(1, 0, "scalar")
                elif d == 4:
                    emit_round(0, 1, "scalar")   # a4, a3
                    emit_round(1, 1, "scalar")
                elif d == 3:
                    emit_round(0, 2, "dve")      # a2, a1
                    emit_round(1, 2, "dve")
                m_ps = acc.tile([P, D], F32, tag=m_tags[d], name=f"m{d}")
                for dpp in range(3):
                    for t in range(2):
                        lhsT = kqt[t][:, 2 * dpp : 2 * dpp + 2, d * P : (d + 1) * P]
                        for off, sz in CHUNKS:
                            nc.tensor.matmul(
                                m_ps[:, off : off + sz], lhsT,
                                w8[:, t, 2 * dpp : 2 * dpp + 2, off : off + sz],
                                start=(t == 0 and dpp == 0),
                                stop=(t == 1 and dpp == 2),
                                perf_mode=DR,
                            )
                nc.scalar.activation(out=msum[:, d, :], in_=m_ps, func=COPY)

            # --- y = x @ Msum; out = y*SO + (x + fb).  Software-pipelined:
            # each tile's cp2/cp1 passes (msum rows 4,5 / 2,3 - drained
            # early) are issued 4 tiles ahead on a 4-deep psum rotation, so
            # only the cp0 passes wait for the final msum row drains ---
            y_tags = ("G3", "G0", "G1", "G2")
            y_ps_l = {}

            def y_mms(i, cps):
                for cp in cps:
                    lhsT = xt8[:, 2 * cp : 2 * cp + 2, i * P : (i + 1) * P]
                    for off, sz in CHUNKS:
                        nc.tensor.matmul(
                            y_ps_l[i][:, off : off + sz], lhsT,
                            msum[:, 2 * cp : 2 * cp + 2, off : off + sz],
                            start=(cp == 2), stop=(cp == 0), perf_mode=DR,
                        )

            for i in range(4):
                y_ps_l[i] = acc.tile([P, D], F32, tag=y_tags[i % 4], name=f"y{i}")
                y_mms(i, (2, 1))
            for i in range(NT):
                y_mms(i, (0,))
                ob = obufp.tile([P, D], BF16, tag="out", name=f"ob{i}")
                if i < NT - 1:
                    nc.vector.scalar_tensor_tensor(
                        out=ob, in0=y_ps_l[i], scalar=SO,
                        in1=xfb[:, i, :], op0=MUL, op1=ADD,
                    )
                    eng = nc.scalar if (i % 2 == 0) else nc.sync
                    eng.dma_start(out=out_t[i], in_=ob)
                else:
                    # last tile: drain per 1-bank chunk to shrink the tail
                    for off, sz in CHUNKS:
                        nc.vector.scalar_tensor_tensor(
                            out=ob[:, off : off + sz],
                            in0=y_ps_l[i][:, off : off + sz], scalar=SO,
                            in1=xfb[:, i, off : off + sz], op0=MUL, op1=ADD,
                        )
                        eng = nc.scalar if off == 0 else nc.sync
                        eng.dma_start(
                            out=out_t[i][:, off : off + sz],
                            in_=ob[:, off : off + sz],
                        )
                if i + 4 < NT:
                    j = i + 4
                    y_ps_l[j] = acc.tile(
                        [P, D], F32, tag=y_tags[j % 4], name=f"y{j}"
                    )
                    y_mms(j, (2, 1))

    nc.compile()
    return nc


def prep_inputs(inputs):
    x = np.asarray(inputs["x"], dtype=np.float32)
    x2 = np.asarray(inputs["x2"], dtype=np.float32)
    x3 = np.asarray(inputs["x3"], dtype=np.float32)
    W1 = np.asarray(inputs["W1"], dtype=np.float32)
    b1 = np.asarray(inputs["b1"], dtype=np.float32)
    W2 = np.asarray(inputs["W2"], dtype=np.float32)
    b2 = np.asarray(inputs["b2"], dtype=np.float32)
    w = np.asarray(inputs["w"], dtype=np.float32)

    f = 1.0 / (1.0 + np.exp(-float(w.reshape(-1)[0])))
    w8 = np.concatenate(
        [(f * CW * W1).T, (f * CW * W2).T], axis=0
    ).astype(NP_FP8)
    fb = (f * (b1 + b2)).astype(np.float32)

    x2_8 = x2.astype(NP_FP8)
    x3_8 = x3.astype(NP_FP8)
    xfb = (x + fb[None, None, :]).astype(NP_BF16)
    ident = np.eye(P, dtype=NP_BF16)
    return [
        {
            "x2": np.ascontiguousarray(x2_8[b]),
            "x3": np.ascontiguousarray(x3_8[b]),
            "xt8": np.ascontiguousarray(x[b].T).astype(NP_FP8),
            "w8": w8,
            "xfb": np.ascontiguousarray(xfb[b]),
            "ident": ident,
        }
        for b in range(B)
    ]


_NC = None


def kernel(**inputs) -> np.ndarray:
    global _NC
    if _NC is None:
        _NC = build_nc()
    in_maps = prep_inputs(inputs)
    res = run_bass_kernel_spmd(_NC, in_maps, list(range(B)))
    return np.stack(
        [res.results[b]["out"] for b in range(B)], axis=0
    ).astype(np.float32)


# revision 38
# speedup vs baseline: 1.0584x; 1.0230x over previous
"""Trainium2 Bass kernel for nn_Cross_Attention (B=8, N=2048, D=768).

Math (per batch b):
    key   = softmax(t, axis=-1).T              (t in {x2, x3})
    query = softmax(t, axis=0)
    attn  = (x @ key^T) @ query = x @ KQ       with KQ [D, D]
    out   = f*(attn_1 @ W1^T + b1) + f*(attn_2 @ W2^T + b2) + x
          = x @ Msum + (x + f*(b1+b2))
    Msum  = f*(KQ_1 @ W1^T + KQ_2 @ W2^T)

KQ[d,d'] = KQ_raw[d,d']/S[d'] with KQ_raw = E^T diag(1/R) E SYMMETRIC,
so only the upper-triangle blocks of KQ_raw are computed (all six row
tiles accumulate in PSUM at once - no post-stream second gram pass; one
accumulation group per 2KB PSUM bank, 8 banks exactly) and the lower
blocks are PE-transposed mirrors of a bf16 staging copy.  Mirror rounds
rotate through the freed gram banks; for t=1 they are woven into the
Msum tile loop (Msum runs d=5..0, needing mirrors latest-first) so Msum
never waits.  Row tile 2 has no room for a ones column (its 513-wide
span would need a 2nd group in the same bank), so its colsum S2 is a
tiny post-gram pass: 8 DR matmuls with a ones lhsT -> S2 as a row
vector, transposed back to a column by one f32 matmul.  All heavy
matmuls run fp8 DoubleRow.

Distribution: pure data-parallel, batch b -> core b, no collectives.
x2/x3 stream 4 tokens per partition line (3 KB descriptors).
"""

import numpy as np
import ml_dtypes

import concourse.bass as bass
import concourse.tile as tile
from concourse import bacc
from concourse import mybir
from concourse.bass_utils import run_bass_kernel_spmd

F32 = mybir.dt.float32
BF16 = mybir.dt.bfloat16
FP8 = mybir.dt.float8e4

NP_FP8 = ml_dtypes.float8_e4m3
NP_BF16 = ml_dtypes.bfloat16

B = 8
P = 128
D = 768
DT = D // P    # 6 feature subtiles
NT = 16        # 128-token tiles
TG = 4         # tokens per partition line / token tiles per DMA group
NG = NT // TG  # 4 stream groups
GD = D + 1     # g1 width: 768 data cols + the ones column
# fp8 prescales (exact powers of two; cancelled in the output scale)
CR = 1024.0    # on E/R   (g1)
CS = 64.0      # on KQ    (kqt)
CST = 16.0     # on raw KQ staging for mirrors
CW = 16.0      # on f*W^T (w8)
SO = 1.0 / (CS * CW)
DR = mybir.MatmulPerfMode.DoubleRow
MUL = mybir.AluOpType.mult
ADD = mybir.AluOpType.add
COPY = mybir.ActivationFunctionType.Copy
EXP = mybir.ActivationFunctionType.Exp

# gram triangle geometry -----------------------------------------------------
# row tile a holds KQ_raw rows [a*128,(a+1)*128), cols [a*128,768) (+ones).
# G psum tiles, one accumulation group per bank:
#   G0 [P,769]: row0 cols 0:512 | cols 512:768 + S0 @768
#   G1 [P,641]: row1 cols 128:640 | cols 640:768 + S1 @640
#   G2 [P,897]: row2 cols 256:768 (no ones) | row3 cols 384:768 + S3 @896
#   G3 [P,641]: row4 cols 512:768 + S4 @256 | row5 cols 640:768 + S5 @640
# stage (bf16 sbuf) packs off-diagonal upper spans (row a, cols >=(a+1)*128):
STG_OFF = (0, 640, 1152, 1664, 1920)
STG_W = (640, 512, 512, 256, 128)   # b=2 spans cols 256:768 incl diagonal
# mirror transpose rounds (post t1-gram): (tag, ((dest row a, offset), ...))
# descending a: Msum runs d=5..0 and needs high-a mirrors first.
X_ROUNDS = (("G1", ((5, 0),)), ("G2", ((4, 0), (3, 512))),
            ("G3", ((2, 0), (1, 256))))


def build_nc():
    N = NT * P
    nc = bacc.Bacc()

    x2_d = nc.dram_tensor("x2", [N, D], FP8, kind="ExternalInput")
    x3_d = nc.dram_tensor("x3", [N, D], FP8, kind="ExternalInput")
    xt8_d = nc.dram_tensor("xt8", [D, N], FP8, kind="ExternalInput")  # x^T
    w8_d = nc.dram_tensor("w8", [2 * D, D], FP8, kind="ExternalInput")
    xfb_d = nc.dram_tensor("xfb", [N, D], BF16, kind="ExternalInput")
    id_d = nc.dram_tensor("ident", [P, P], BF16, kind="ExternalInput")
    out_d = nc.dram_tensor("out", [N, D], BF16, kind="ExternalOutput")

    # token n = g*512 + p*4 + j  ->  3 KB contiguous partition lines
    att_g = [
        x2_d.rearrange("(g p j) d -> g p j d", p=P, j=TG),
        x3_d.rearrange("(g p j) d -> g p j d", p=P, j=TG),
    ]
    xt8_r = xt8_d.rearrange("(c p) n -> p c n", p=P)
    w8_r = w8_d.rearrange("(t c p) j -> p t c j", p=P, c=DT)
    xfb_r = xfb_d.rearrange("(h t p) d -> h p t d", p=P, t=NT // 2)
    out_t = out_d.rearrange("(t p) d -> t p d", p=P)

    with tile.TileContext(nc) as tc:
        with (
            tc.tile_pool(name="consts", bufs=1) as consts,
            tc.tile_pool(name="gbuf", bufs=2) as gbuf,
            tc.tile_pool(name="stream", bufs=3) as stream,
            tc.tile_pool(name="stats", bufs=2) as stats,
            tc.tile_pool(name="obuf", bufs=6) as obufp,
            tc.tile_pool(name="acc", bufs=1, space="PSUM") as acc,
        ):
            ones = consts.tile([P, 2, P], FP8)
            nc.vector.memset(ones, 1.0)
            onef = consts.tile([1, 1], F32)
            nc.vector.memset(onef, 1.0)
            ident = consts.tile([P, P], BF16)
            kqt = [consts.tile([P, DT, D], FP8, name=f"kqt{t}") for t in range(2)]
            msum = consts.tile([P, DT, D], FP8)
            xt8 = consts.tile([P, DT, N], FP8)
            w8 = consts.tile([P, 2, DT, D], FP8)
            xfb = consts.tile([P, NT, D], BF16)
            mpart = consts.tile([P, DT, D], F32, name="mpart")

            # probe tiles for the scalar-engine dtype-rate experiment
            pf8 = consts.tile([P, 2], FP8)
            po8 = consts.tile([P, 2], FP8, name="po8")
            nc.vector.memset(pf8, 0.25)
            # tiny exp at t=0: pulls the ACT table load into the startup gap
            nc.scalar.activation(out=po8[:, 0:1], in_=pf8[:, 0:1], func=EXP)

            # PE warmup (p-state ramp); rides in the G0 psum slot
            warm = acc.tile([P, GD], F32, tag="G0", name="warm")
            for _ in range(16):
                nc.tensor.matmul(
                    warm[:, 0:P], ones, ones, start=True, stop=True, perf_mode=DR
                )

            # --- input DMAs: queue the whole schedule on the sync engine ---
            xi_t = {}
            for t in range(2):
                for g in range(NG):
                    xi = stream.tile([P, TG, D], FP8, tag="in", name=f"xi{t}_{g}")
                    xi_t[(t, g)] = xi
                    if t == 0 and g == 0:
                        nc.sync.dma_start(out=xi[:, 0:1, :], in_=att_g[t][g][:, 0:1, :])
                        nc.sync.dma_start(out=xi[:, 1:2, :], in_=att_g[t][g][:, 1:2, :])
                        nc.sync.dma_start(out=xi[:, 2:4, :], in_=att_g[t][g][:, 2:4, :])
                        nc.sync.dma_start(out=ident, in_=id_d[:, :])
                    else:
                        nc.sync.dma_start(out=xi, in_=att_g[t][g])
            nc.sync.dma_start(out=w8, in_=w8_r)
            nc.sync.dma_start(out=xt8, in_=xt8_r)
            nc.sync.dma_start(out=xfb[:, 0:8, :], in_=xfb_r[0])
            nc.sync.dma_start(out=xfb[:, 8:16, :], in_=xfb_r[1])

            # --- per-t state ---
            st = {}

            def emit_stream_group(t, g, skip_g3=False):
                """exp + g1 scale + triangle gram passes for stream group g."""
                if g == 0:
                    d = {}
                    d["g2"] = gbuf.tile([P, NT, D], FP8, tag="g2", name=f"g2_{t}")
                    d["g1"] = gbuf.tile([P, NT, GD], FP8, tag="g1", name=f"g1_{t}")
                    nc.gpsimd.memset(d["g1"][:, :, D:GD], 1.0)
                    d["rv"] = stats.tile([P, NT], F32, tag="rvec", name=f"rv{t}")
                    d["rvr"] = stats.tile([P, NT], F32, tag="rvr", name=f"rvr{t}")
                    d["srR"] = stats.tile([P, DT], F32, tag="srR", name=f"srR{t}")
                    d["srA"] = stats.tile([P, DT], F32, tag="srA", name=f"srA{t}")
                    d["srB"] = stats.tile([P, DT], F32, tag="srB", name=f"srB{t}")
                    d["stg"] = gbuf.tile([P, 2048], BF16, tag="stage",
                                         name=f"stg{t}")
                    d["s2r"] = stats.tile([1, P], F32, tag="s2r", name=f"s2r{t}")
                    d["G"] = [
                        acc.tile([P, GD], F32, tag="G0", name=f"G0_{t}"),
                        acc.tile([P, 641], F32, tag="G1", name=f"G1_{t}"),
                        acc.tile([P, 897], F32, tag="G2", name=f"G2_{t}"),
                    ]
                    if not skip_g3:
                        d["G3"] = acc.tile([P, 641], F32, tag="G3",
                                           name=f"G3_{t}")
                    st[t] = d
                d = st[t]
                g1, g2, rvec, rvr = d["g1"], d["g2"], d["rv"], d["rvr"]
                xi = xi_t[(t, g)]
                for j in range(TG):
                    i = g * TG + j
                    nc.scalar.activation(
                        out=g2[:, i, :], in_=xi[:, j, :], func=EXP,
                        accum_out=rvec[:, i : i + 1],
                    )
                for h in range(2):
                    i0 = g * TG + 2 * h
                    nc.vector.reciprocal(
                        rvr[:, i0 : i0 + 2], rvec[:, i0 : i0 + 2]
                    )
                    for i in (i0, i0 + 1):
                        nc.vector.tensor_scalar(
                            out=g1[:, i, 0:D], in0=g2[:, i, :],
                            scalar1=rvr[:, i : i + 1], scalar2=CR,
                            op0=MUL, op1=MUL,
                        )
                G0, G1t, G2t = d["G"]
                G3t = d.get("G3")
                for half in range(2):
                    ip = 2 * g + half
                    pr = slice(4 * g + 2 * half, 4 * g + 2 * half + 2)
                    s0, s1 = (ip == 0), (ip == 7)

                    def mm(out, c0, c1, r0, r1):
                        nc.tensor.matmul(
                            out, g2[:, pr, c0:c1], g1[:, pr, r0:r1],
                            start=s0, stop=s1, perf_mode=DR,
                        )

                    mm(G0[:, 0:512], 0, 128, 0, 512)
                    mm(G0[:, 512:769], 0, 128, 512, 769)
                    mm(G1t[:, 0:512], 128, 256, 128, 640)
                    mm(G1t[:, 512:641], 128, 256, 640, 769)
                    mm(G2t[:, 0:512], 256, 384, 256, 768)
                    mm(G2t[:, 512:897], 384, 512, 384, 769)
                    if not skip_g3:
                        mm(G3t[:, 0:257], 512, 640, 512, 769)
                        mm(G3t[:, 512:641], 640, 768, 640, 769)

            # (S col, scaled-drain src, kqt row a, kqt col, stage idx or None)
            def plans(t):
                G0, G1t, G2t = st[t]["G"]
                G3t = st[t]["G3"]
                return [
                    (G0[:, 768:769], G0[:, 0:D], 0, 0, 0),
                    (G1t[:, 640:641], G1t[:, 0:640], 1, 128, 1),
                    (None, G2t[:, 0:512], 2, 256, 2),
                    (G2t[:, 896:897], G2t[:, 512:896], 3, 384, 3),
                    (G3t[:, 256:257], G3t[:, 0:256], 4, 512, 4),
                    (G3t[:, 640:641], G3t[:, 512:640], 5, 640, None),
                ]

            def stage_src(t, b):
                G0, G1t, G2t = st[t]["G"]
                G3t = st[t]["G3"]
                return [G0[:, 128:768], G1t[:, 128:640], G2t[:, 0:512],
                        G2t[:, 640:896], G3t[:, 128:256]][b]

            def emit_stage(t, b, eng="dve"):
                dst = st[t]["stg"][:, STG_OFF[b] : STG_OFF[b] + STG_W[b]]
                if eng == "dve":
                    nc.vector.tensor_scalar(
                        out=dst, in0=stage_src(t, b),
                        scalar1=CST / CR, scalar2=1.0, op0=MUL, op1=MUL,
                    )
                elif eng == "gpsimd":
                    nc.gpsimd.tensor_scalar(
                        out=dst, in0=stage_src(t, b),
                        scalar1=CST / CR, scalar2=1.0, op0=MUL, op1=MUL,
                    )
                else:
                    nc.scalar.activation(
                        out=dst, in_=stage_src(t, b), func=COPY, scale=CST / CR
                    )

            def emit_row(t, k, eng, do_stage=True):
                """recip + scaled drain (+ stage drain) for row k."""
                d = st[t]
                scol, src, a, c0, b = plans(t)[k]
                if scol is not None:
                    nc.vector.reciprocal(d["srR"][:, k : k + 1], scol)
                w = src.shape[-1]
                dst = kqt[t][:, a, c0 : c0 + w]
                if eng == "dve":
                    nc.vector.tensor_scalar(
                        out=dst, in0=src, scalar1=d["srR"][:, k : k + 1],
                        scalar2=CS / CR, op0=MUL, op1=MUL,
                    )
                elif eng == "gpsimd":
                    nc.gpsimd.tensor_scalar(
                        out=dst, in0=src, scalar1=d["srR"][:, k : k + 1],
                        scalar2=CS / CR, op0=MUL, op1=MUL,
                    )
                else:
                    nc.vector.tensor_scalar(
                        out=d["srA"][:, k : k + 1], in0=d["srR"][:, k : k + 1],
                        scalar1=CS / CR, scalar2=1.0, op0=MUL, op1=MUL,
                    )
                    nc.scalar.activation(
                        out=dst, in_=src, func=COPY,
                        scale=d["srA"][:, k : k + 1],
                    )
                if b is not None and do_stage:
                    emit_stage(t, b)

            def emit_g3_gram(t):
                d = st[t]
                d["G3"] = acc.tile([P, 641], F32, tag="G3", name=f"G3_{t}")
                G3t = d["G3"]
                g1, g2 = d["g1"], d["g2"]
                for ip in range(8):
                    pr = slice(2 * ip, 2 * ip + 2)
                    s0, s1 = (ip == 0), (ip == 7)
                    nc.tensor.matmul(
                        G3t[:, 0:257], g2[:, pr, 512:640], g1[:, pr, 512:769],
                        start=s0, stop=s1, perf_mode=DR,
                    )
                    nc.tensor.matmul(
                        G3t[:, 512:641], g2[:, pr, 640:768], g1[:, pr, 640:769],
                        start=s0, stop=s1, perf_mode=DR,
                    )

            def emit_mt0(dd, eng):
                # Msum t0-half for output tile dd -> f32 partial in SBUF
                mt_ps = acc.tile([P, D], F32, tag="G3", name=f"mt{dd}")
                for dpp in range(3):
                    lhsT = kqt[0][:, 2 * dpp : 2 * dpp + 2,
                                  dd * P : (dd + 1) * P]
                    for off, sz in ((0, 512), (512, 256)):
                        nc.tensor.matmul(
                            mt_ps[:, off : off + sz], lhsT,
                            w8[:, 0, 2 * dpp : 2 * dpp + 2, off : off + sz],
                            start=(dpp == 0), stop=(dpp == 2), perf_mode=DR,
                        )
                e = nc.vector if eng == "dve" else nc.gpsimd
                e.tensor_copy(mpart[:, dd, :], mt_ps)

            def emit_row2_final(t, eng="dve"):
                # kqt row2 upper from the bf16 stage (needs 1/S2 from s2 pass)
                d = st[t]
                e = nc.vector if eng == "dve" else nc.gpsimd
                e.tensor_scalar(
                    out=kqt[t][:, 2, 256:D],
                    in0=d["stg"][:, STG_OFF[2] : STG_OFF[2] + 512],
                    scalar1=d["srR"][:, 2:3], scalar2=CS / CST,
                    op0=MUL, op1=MUL,
                )

            def emit_s2_pass(t):
                """row2 colsum: ones^T @ g2 cols -> s2 row; transpose; recip."""
                d = st[t]
                s2_ps = acc.tile([1, P], F32, tag="G0", name=f"s2ps{t}")
                g2 = d["g2"]
                for ip in range(8):
                    pr = slice(2 * ip, 2 * ip + 2)
                    nc.tensor.matmul(
                        s2_ps, ones[:, :, 0:1], g2[:, pr, 256:384],
                        start=(ip == 0), stop=(ip == 7), perf_mode=DR,
                    )
                nc.vector.tensor_copy(d["s2r"], s2_ps)
                s2c = acc.tile([P, 1], F32, tag="G0", name=f"s2c{t}")
                nc.tensor.matmul(s2c, d["s2r"], onef, start=True, stop=True)
                nc.vector.reciprocal(d["srR"][:, 2:3], s2c)

            def emit_round(t, r, eng):
                """transpose round r + its mirror drains."""
                d = st[t]
                tag, blocks = X_ROUNDS[r]
                X = acc.tile([P, 1024], BF16, tag=tag, name=f"X{t}_{r}")
                stg = d["stg"]
                for a, xoff in blocks:
                    for b in range(a):
                        # span b starts at col (b+1)*128, except b=2 at col 256
                        s0 = STG_OFF[b] + (a - b - 1) * P + (P if b == 2 else 0)
                        nc.tensor.transpose(
                            X[:, xoff + b * P : xoff + (b + 1) * P],
                            stg[:, s0 : s0 + P], ident,
                        )
                for a, xoff in blocks:
                    dst = kqt[t][:, a, 0 : a * P]
                    src = X[:, xoff : xoff + a * P]
                    if eng == "gpsimd":
                        nc.gpsimd.tensor_scalar(
                            out=dst, in0=src, scalar1=d["srR"][:, a : a + 1],
                            scalar2=CS / CST, op0=MUL, op1=MUL,
                        )
                    elif eng == "dve":
                        nc.vector.tensor_scalar(
                            out=dst, in0=src, scalar1=d["srR"][:, a : a + 1],
                            scalar2=CS / CST, op0=MUL, op1=MUL,
                        )
                    else:
                        nc.vector.tensor_scalar(
                            out=d["srB"][:, a : a + 1],
                            in0=d["srR"][:, a : a + 1],
                            scalar1=CS / CST, scalar2=1.0, op0=MUL, op1=MUL,
                        )
                        nc.scalar.activation(
                            out=dst, in_=src, func=COPY,
                            scale=d["srB"][:, a : a + 1],
                        )

            # ------------------- schedule -------------------
            for g in range(NG):
                emit_stream_group(0, g)
            # t0 drains (DVE) woven with t1's stream groups: free banks in
            # tag order G0..G3 so t1's gram mms unblock in emission order
            emit_row(0, 0, "dve")
            emit_stream_group(1, 0)
            emit_row(0, 1, "dve")
            emit_stream_group(1, 1)
            emit_stage(0, 2)
            emit_row(0, 3, "dve")
            emit_stream_group(1, 2)
            emit_row(0, 4, "dve")
            emit_row(0, 5, "dve")
            emit_stream_group(1, 3)

            # t1 drains: rows on scalar (free post-exp), stages on DVE
            emit_row(1, 0, "scalar")
            emit_row(1, 1, "scalar")
            emit_stage(1, 2)
            emit_row(1, 3, "scalar")
            emit_row(1, 4, "scalar")
            emit_row(1, 5, "scalar")

            # s2 colsum passes (bank G0 freed first), then row2 finals
            emit_s2_pass(0)
            emit_s2_pass(1)
            emit_row2_final(0)
            emit_row2_final(1)

            # --- Msum, output tiles d = 5..0 (mirror-light first); mirror
            # rounds woven in so M_d never waits on a mirror ---
            CHUNKS = ((0, 512), (512, 256))
            m_tags = {5: "G0", 4: "G1", 3: "G2", 2: "G0", 1: "G1", 0: "G2"}
            for d in (5, 4, 3, 2, 1, 0):
                if d == 5:
                    emit_round(0, 0, "scalar")   # a5 mirrors
                    emit_round(1, 0, "scalar")
                elif d == 4:
                    emit_round(0, 1, "scalar")   # a4, a3
                    emit_round(1, 1, "scalar")
                elif d == 3:
                    emit_round(0, 2, "dve")      # a2, a1
                    emit_round(1, 2, "dve")
                m_ps = acc.tile([P, D], F32, tag=m_tags[d], name=f"m{d}")
                for dpp in range(3):
                    for t in range(2):
                        lhsT = kqt[t][:, 2 * dpp : 2 * dpp + 2, d * P : (d + 1) * P]
                        for off, sz in CHUNKS:
                            nc.tensor.matmul(
                                m_ps[:, off : off + sz], lhsT,
                                w8[:, t, 2 * dpp : 2 * dpp + 2, off : off + sz],
                                start=(t == 0 and dpp == 0),
                                stop=(t == 1 and dpp == 2),
                                perf_mode=DR,
                            )
                if d == 0:
                    nc.scalar.activation(
                        out=msum[:, d, 0:512], in_=m_ps[:, 0:512], func=COPY
                    )
                    nc.vector.tensor_scalar(
                        out=msum[:, d, 512:D], in0=m_ps[:, 512:D],
                        scalar1=1.0, scalar2=1.0, op0=MUL, op1=MUL,
                    )
                else:
                    nc.scalar.activation(out=msum[:, d, :], in_=m_ps, func=COPY)

            # --- y = x @ Msum; out = y*SO + (x + fb) ---
            y_tags = ("G0", "G1", "G2")
            for i in range(NT):
                y_ps = acc.tile([P, D], F32, tag=y_tags[i % 3], name=f"y{i}")
                ob = obufp.tile([P, D], BF16, tag="out", name=f"ob{i}")
                if i < NT - 1:
                    for cp in (2, 1, 0):
                        lhsT = xt8[:, 2 * cp : 2 * cp + 2, i * P : (i + 1) * P]
                        for off, sz in CHUNKS:
                            nc.tensor.matmul(
                                y_ps[:, off : off + sz], lhsT,
                                msum[:, 2 * cp : 2 * cp + 2, off : off + sz],
                                start=(cp == 2), stop=(cp == 0), perf_mode=DR,
                            )
                    nc.vector.scalar_tensor_tensor(
                        out=ob, in0=y_ps, scalar=SO,
                        in1=xfb[:, i, :], op0=MUL, op1=ADD,
                    )
                    eng = nc.scalar if (i % 2 == 0) else nc.sync
                    eng.dma_start(out=out_t[i], in_=ob)
                else:
                    # last tile: drain per 1-bank chunk to shrink the tail
                    for off, sz in CHUNKS:
                        for cp in (2, 1, 0):
                            lhsT = xt8[:, 2 * cp : 2 * cp + 2, i * P : (i + 1) * P]
                            nc.tensor.matmul(
                                y_ps[:, off : off + sz], lhsT,
                                msum[:, 2 * cp : 2 * cp + 2, off : off + sz],
                                start=(cp == 2), stop=(cp == 0), perf_mode=DR,
                            )
                        nc.vector.scalar_tensor_tensor(
                            out=ob[:, off : off + sz], in0=y_ps[:, off : off + sz],
                            scalar=SO, in1=xfb[:, i, off : off + sz],
                            op0=MUL, op1=ADD,
                        )
                        eng = nc.scalar if off == 0 else nc.sync
                        eng.dma_start(
                            out=out_t[i][:, off : off + sz],
                            in_=ob[:, off : off + sz],
                        )

    nc.compile()
    return nc


def prep_inputs(inputs):
    x = np.asarray(inputs["x"], dtype=np.float32)
    x2 = np.asarray(inputs["x2"], dtype=np.float32)
    x3 = np.asarray(inputs["x3"], dtype=np.float32)
    W1 = np.asarray(inputs["W1"], dtype=np.float32)
    b1 = np.asarray(inputs["b1"], dtype=np.float32)
    W2 = np.asarray(inputs["W2"], dtype=np.float32)
    b2 = np.asarray(inputs["b2"], dtype=np.float32)
    w = np.asarray(inputs["w"], dtype=np.float32)

    f = 1.0 / (1.0 + np.exp(-float(w.reshape(-1)[0])))
    w8 = np.concatenate(
        [(f * CW * W1).T, (f * CW * W2).T], axis=0
    ).astype(NP_FP8)
    fb = (f * (b1 + b2)).astype(np.float32)

    x2_8 = x2.astype(NP_FP8)
    x3_8 = x3.astype(NP_FP8)
    xfb = (x + fb[None, None, :]).astype(NP_BF16)
    ident = np.eye(P, dtype=NP_BF16)
    return [
        {
            "x2": np.ascontiguousarray(x2_8[b]),
            "x3": np.ascontiguousarray(x3_8[b]),
            "xt8": np.ascontiguousarray(x[b].T).astype(NP_FP8),
            "w8": w8,
            "xfb": np.ascontiguousarray(xfb[b]),
            "ident": ident,
        }
        for b in range(B)
    ]


_NC = None


def kernel(**inputs) -> np.ndarray:
    global _NC
    if _NC is None:
        _NC = build_nc()
    in_maps = prep_inputs(inputs)
    res = run_bass_kernel_spmd(_NC, in_maps, list(range(B)))
    return np.stack(
        [res.results[b]["out"] for b in range(B)], axis=0
    ).astype(np.float32)
